# revision 11
# baseline (speedup 1.0000x reference)
"""HEX loss kernel for Trainium2 (8 NeuronCores, batch-parallel, raw Bass).

Math: the chain junction-tree distribution factorizes into independent
Bernoullis with P(y_v=1) = sigmoid(fs[b,v]); hence
    loss = mean_b softplus(-fs[b, labels[b]])

Implementation: only fs[b, labels[b]] matters (4 B per row). Each core
runs one SWDGE dma_gather of 4096 x 256 B chunks (the 64-float block of
each row containing its label; idx = 4*i + lab>>6, host-computed), an
additive -inf mask (host-built) + grouped reduce_max picks the target
float, then softplus(-sel) = Ln(1 + Exp(-sel)) on ACT with the free-dim
sum taken by the activation accumulator. Host sums 8x128 partials / B.
"""

import numpy as np

B = 32768
V = 256
N_CORES = 8
BL = B // N_CORES   # 4096 rows per core
P = 128
S = BL // P         # 32 rows per partition
K = 64              # floats per gathered chunk (256 B)
NEG = -1.0e30

_CACHE = {}


def _build():
    from contextlib import ExitStack

    import concourse.bass as bass  # noqa
    import concourse.tile as tile  # noqa
    from concourse import bacc, mybir

    f32 = mybir.dt.float32
    bf16 = mybir.dt.bfloat16
    i16 = mybir.dt.int16
    Act = mybir.ActivationFunctionType
    Alu = mybir.AluOpType

    nc = bacc.Bacc(
        "TRN2",
        target_bir_lowering=False,
        debug=False,
        enable_asserts=False,
        num_devices=N_CORES,
    )

    # fs bytes viewed as bf16 so the gather matches the ucode-tested dtype;
    # each 256 B chunk is 128 "bf16" elements = 64 real f32s.
    fs_d = nc.dram_tensor("fs", [BL * V // K, 2 * K], bf16, kind="ExternalInput").ap()
    idx_d = nc.dram_tensor("idx", [P, BL // 16], i16, kind="ExternalInput").ap()
    msk_d = nc.dram_tensor("msk", [P, S * K], f32, kind="ExternalInput").ap()
    out_d = nc.dram_tensor("out", [P, 1], f32, kind="ExternalOutput").ap()

    with ExitStack() as ctx:
        idxs = ctx.enter_context(nc.sbuf_tensor([P, BL // 16], i16))
        msk = ctx.enter_context(nc.sbuf_tensor([P, S * K], f32))
        gath = ctx.enter_context(nc.sbuf_tensor([P, 2 * S * K], bf16))
        t = ctx.enter_context(nc.sbuf_tensor([P, S * K], bf16))
        sel = ctx.enter_context(nc.sbuf_tensor([P, S], f32))
        u = ctx.enter_context(nc.sbuf_tensor([P, S], f32))
        y = ctx.enter_context(nc.sbuf_tensor([P, S], f32))
        acc = ctx.enter_context(nc.sbuf_tensor([P, 1], f32))

        sem_idx = ctx.enter_context(nc.semaphore("s_idx"))
        sem_msk = ctx.enter_context(nc.semaphore("s_msk"))
        sem_g = ctx.enter_context(nc.semaphore("s_g"))
        sem_sel = ctx.enter_context(nc.semaphore("s_sel"))
        sem_acc = ctx.enter_context(nc.semaphore("s_acc"))
        sem_out = ctx.enter_context(nc.semaphore("s_out"))

        blk = ctx.enter_context(nc.Block())

        @blk.sync
        def _(s_eng):
            s_eng.dma_start(out=idxs.ap(), in_=idx_d).then_inc(sem_idx, 16)
            s_eng.dma_start(out=msk.ap(), in_=msk_d).then_inc(sem_msk, 16)

        @blk.gpsimd
        def _(g_eng):
            g_eng.wait_ge(sem_idx, 16)
            g_eng.dma_gather(
                out_ap=gath.ap().rearrange("p (s k) -> p s k", k=2 * K),
                in_ap=fs_d,
                idxs_ap=idxs.ap(),
                num_idxs=BL,
                num_idxs_reg=BL,
                elem_size=2 * K,
                single_packet=False,
            ).then_inc(sem_g, 16)

        @blk.vector
        def _(v_eng):
            v_eng.wait_ge(sem_g, 16)
            v_eng.wait_ge(sem_msk, 16)
            v_eng.tensor_add(t.ap(), gath.ap().bitcast(f32), msk.ap())
            v_eng.drain()
            v_eng.tensor_reduce(
                sel.ap(),
                t.ap().rearrange("p (s k) -> p s k", k=K),
                axis=mybir.AxisListType.X,
                op=Alu.max,
            ).then_inc(sem_sel, 1)

        @blk.scalar
        def _(a_eng):
            from concourse.hw_specs import get_activation_tables

            tabs = list(get_activation_tables(nc.m.arch).items())
            tid = next(
                i for i, (n, s) in enumerate(tabs) if Act.Exp in s and Act.Ln in s
            )
            a_eng.add_instruction(
                mybir.InstLoadActFuncSet(
                    name=nc.get_next_instruction_name(),
                    ins=[],
                    outs=[],
                    act_func_set_id=tid,
                )
            )
            a_eng.wait_ge(sem_sel, 1)
            a_eng.activation(u.ap(), sel.ap(), Act.Exp, scale=-1.0)
            a_eng.drain()
            a_eng.activation(y.ap(), u.ap(), Act.Ln, bias=1.0, accum_out=acc.ap())
            a_eng.drain()
            # No wait on sem_out: the ~8us HBM-write receipt would sit on the
            # critical path; the runtime quiesces DMA queues at NEFF end.
            a_eng.dma_start(out=out_d, in_=acc.ap()).then_inc(sem_out, 16)

    nc.compile()
    return nc


def _get_nc():
    if "nc" not in _CACHE:
        _CACHE["nc"] = _build()
    return _CACHE["nc"]


def _shard_inputs(fs, labels):
    import ml_dtypes

    fs = np.ascontiguousarray(np.asarray(fs, dtype=np.float32))
    labels = np.asarray(labels).astype(np.int64)
    rows4 = np.arange(BL, dtype=np.int64) * (V // K)
    in_maps = []
    for c in range(N_CORES):
        fs_loc = (
            fs[c * BL : (c + 1) * BL]
            .reshape(BL * V // K, K)
            .view(ml_dtypes.bfloat16)
        )
        lab_loc = labels[c * BL : (c + 1) * BL]
        # gather position j = batch row i; idx into 64-float rows of fs
        idx = (rows4 + (lab_loc >> 6)).astype(np.int16)
        # ucode layout: idx j read from partition j%16, column j//16; replicate x8
        idx16 = np.tile(idx.reshape(BL // 16, 16).T, (8, 1))
        # gather lands row i at [i%128, i//128]; mask selects lab%64 within chunk
        m = (lab_loc & 63).reshape(S, P).T  # [P, S]
        msk = np.full((P, S, K), NEG, dtype=np.float32)
        pp, ss = np.meshgrid(np.arange(P), np.arange(S), indexing="ij")
        msk[pp, ss, m] = 0.0
        in_maps.append(
            {
                "fs": fs_loc,
                "idx": np.ascontiguousarray(idx16),
                "msk": msk.reshape(P, S * K),
            }
        )
    return in_maps


def kernel(fs, labels, _trace=False, _trace_kwargs=None):
    from concourse.bass_utils import run_bass_kernel_spmd

    nc = _get_nc()
    in_maps = _shard_inputs(fs, labels)
    res = run_bass_kernel_spmd(
        nc,
        in_maps,
        core_ids=list(range(N_CORES)),
        trace=_trace,
        **(_trace_kwargs or {}),
    )
    total = np.float64(0.0)
    for c in range(N_CORES):
        total += res.results[c]["out"].astype(np.float64).sum()
    loss = total / np.float64(B)
    if _trace:
        return np.float64(loss), res
    return np.asarray(loss, dtype=np.float64)


# revision 15
# speedup vs baseline: 2.8349x; 2.8349x over previous
"""HEX loss kernel for Trainium2 (8 NeuronCores, batch-parallel, raw Bass).

Math: the chain junction-tree distribution factorizes into independent
Bernoullis with P(y_v=1) = sigmoid(fs[b,v]); hence
    loss = mean_b softplus(-fs[b, labels[b]])

Implementation: only fs[b, labels[b]] matters. Rows are assigned to
cores/slots on the host so that slot s = q*C + j holds a row whose
label lies in 32-column block q (8 blocks, C=640 slots each, padded
with duplicate rows). The device then needs just ONE diagonal strided
SWDGE cast-DMA that reads each slot's 32-float block (0.64 MB instead
of 4 MB per core), an additive host-built mask (+BIG on pad slots so
they contribute softplus=0, 0 at the label column, -BIG elsewhere) +
grouped reduce_max to pick the target, and softplus(-sel) =
Ln(1 + Exp(-sel)) on ACT with the free-dim sum from the activation
accumulator. The output store is not waited on (runtime quiesces DMA
queues at NEFF end; an explicit wait costs ~8 us of HBM-write receipt).
Host sums the 8x128 partials / B.
"""

import numpy as np

B = 32768
V = 256
N_CORES = 8
BL = B // N_CORES   # 4096 rows per core
P = 128
K = 32              # floats per block read per row
NQ = V // K         # 8 column blocks
C = 640             # padded slots per (core, block); 5*128
G = C // P          # 5
SLOTS = NQ * C      # 5120 slots per core
W = SLOTS * K // P  # 1280 free-dim elems per partition
SEL = SLOTS // P    # 40 selected values per partition
BIG = 1.0e30

_CACHE = {}


def _build():
    from contextlib import ExitStack

    import concourse.bass as bass
    import concourse.tile as tile  # noqa
    from concourse import bacc, mybir

    f32 = mybir.dt.float32
    bf16 = mybir.dt.bfloat16
    Act = mybir.ActivationFunctionType
    Alu = mybir.AluOpType

    nc = bacc.Bacc(
        "TRN2",
        target_bir_lowering=False,
        debug=False,
        enable_asserts=False,
        num_devices=N_CORES,
    )

    fsp_d = nc.dram_tensor("fsp", [SLOTS, V], f32, kind="ExternalInput").ap()
    msk_d = nc.dram_tensor("msk", [P, W], bf16, kind="ExternalInput").ap()
    out_d = nc.dram_tensor("out", [P, 1], f32, kind="ExternalOutput").ap()

    # diagonal views, one per g: element (p, q, k) = fsp[q*C + g*P + p, K*q + k]
    # (the DMA AP balancer allows at most 3 dims, so the g axis is unrolled)
    fs_diag = [
        bass.AP(
            fsp_d.tensor,
            g * P * V,
            [[V, P], [C * V + K, NQ], [1, K]],
        )
        for g in range(G)
    ]

    with ExitStack() as ctx:
        msk = ctx.enter_context(nc.sbuf_tensor([P, W], bf16))
        gath = ctx.enter_context(nc.sbuf_tensor([P, W], bf16))
        t = ctx.enter_context(nc.sbuf_tensor([P, W], bf16))
        sel = ctx.enter_context(nc.sbuf_tensor([P, SEL], f32))
        u = ctx.enter_context(nc.sbuf_tensor([P, SEL], f32))
        y = ctx.enter_context(nc.sbuf_tensor([P, SEL], f32))
        acc = ctx.enter_context(nc.sbuf_tensor([P, 1], f32))

        sem_g = ctx.enter_context(nc.semaphore("s_g"))
        sem_m = ctx.enter_context(nc.semaphore("s_m"))
        sem_sel = ctx.enter_context(nc.semaphore("s_sel"))
        sem_out = ctx.enter_context(nc.semaphore("s_out"))

        blk = ctx.enter_context(nc.Block())

        @blk.sync
        def _(s_eng):
            s_eng.dma_start(out=msk.ap(), in_=msk_d).then_inc(sem_m, 16)

        @blk.gpsimd
        def _(g_eng):
            gview = gath.ap().rearrange("p (q g k) -> p q g k", q=NQ, k=K)
            for g in range(G):
                g_eng.dma_start(
                    out=gview[:, :, g, :], in_=fs_diag[g]
                ).then_inc(sem_g, 16)

        @blk.vector
        def _(v_eng):
            v_eng.wait_ge(sem_g, 16 * G)
            v_eng.wait_ge(sem_m, 16)
            v_eng.tensor_add(t.ap(), gath.ap(), msk.ap())
            v_eng.drain()
            v_eng.tensor_reduce(
                sel.ap(),
                t.ap().rearrange("p (c k) -> p c k", k=K),
                axis=mybir.AxisListType.X,
                op=Alu.max,
            ).then_inc(sem_sel, 1)

        @blk.scalar
        def _(a_eng):
            from concourse.hw_specs import get_activation_tables

            tabs = list(get_activation_tables(nc.m.arch).items())
            tid = next(
                i for i, (n, s) in enumerate(tabs) if Act.Exp in s and Act.Ln in s
            )
            a_eng.add_instruction(
                mybir.InstLoadActFuncSet(
                    name=nc.get_next_instruction_name(),
                    ins=[],
                    outs=[],
                    act_func_set_id=tid,
                )
            )
            a_eng.wait_ge(sem_sel, 1)
            a_eng.activation(u.ap(), sel.ap(), Act.Exp, scale=-1.0)
            a_eng.drain()
            a_eng.activation(y.ap(), u.ap(), Act.Ln, bias=1.0, accum_out=acc.ap())
            a_eng.drain()
            # no wait on sem_out: the ~8us HBM-write receipt would sit on the
            # critical path; the runtime quiesces DMA queues at NEFF end.
            a_eng.dma_start(out=out_d, in_=acc.ap()).then_inc(sem_out, 16)

    nc.compile()
    return nc


def _get_nc():
    if "nc" not in _CACHE:
        _CACHE["nc"] = _build()
    return _CACHE["nc"]


def _shard_inputs(fs, labels):
    import ml_dtypes

    fs = np.ascontiguousarray(np.asarray(fs, dtype=np.float32))
    labels = np.asarray(labels).astype(np.int64)
    q_all = labels >> 5          # column block of each row
    kk_all = labels & (K - 1)    # position within the block

    # Assign rows to (core, block-bucket) with global balancing: rows of each
    # block q are dealt round-robin across cores, so every (core, q) bucket
    # holds <= ceil(count_q / 8) <= C rows.
    order = np.argsort(q_all, kind="stable")
    counts = np.bincount(q_all, minlength=NQ)
    assert counts.max() <= C * N_CORES, counts
    rows_by_cq = [[[] for _ in range(NQ)] for _ in range(N_CORES)]
    pos = 0
    for q in range(NQ):
        rows_q = order[pos : pos + counts[q]]
        pos += counts[q]
        for i, r in enumerate(rows_q):
            rows_by_cq[i % N_CORES][q].append(r)

    in_maps = []
    for c in range(N_CORES):
        slot_rows = np.zeros(SLOTS, dtype=np.int64)
        pad = np.ones(SLOTS, dtype=bool)
        for q in range(NQ):
            rows = rows_by_cq[c][q]
            n = len(rows)
            assert n <= C, (c, q, n)
            slot_rows[q * C : q * C + n] = rows
            pad[q * C : q * C + n] = False
        fsp = fs[slot_rows]  # [SLOTS, V]

        # mask in slot layout -> [P, W]: slot s = q*C + g*P + p maps to
        # partition p, free offset q*(G*K) + g*K
        kk = kk_all[slot_rows]
        m = np.full((SLOTS, K), -BIG, dtype=np.float32)
        m[np.arange(SLOTS), kk] = 0.0
        m[pad] = BIG
        msk = (
            m.reshape(NQ, G, P, K)
            .transpose(2, 0, 1, 3)
            .reshape(P, W)
            .astype(ml_dtypes.bfloat16)
        )
        in_maps.append(
            {"fsp": fsp, "msk": np.ascontiguousarray(msk)}
        )
    return in_maps


def kernel(fs, labels, _trace=False, _trace_kwargs=None):
    from concourse.bass_utils import run_bass_kernel_spmd

    nc = _get_nc()
    in_maps = _shard_inputs(fs, labels)
    res = run_bass_kernel_spmd(
        nc,
        in_maps,
        core_ids=list(range(N_CORES)),
        trace=_trace,
        **(_trace_kwargs or {}),
    )
    total = np.float64(0.0)
    for c in range(N_CORES):
        total += res.results[c]["out"].astype(np.float64).sum()
    loss = total / np.float64(B)
    if _trace:
        return np.float64(loss), res
    return np.asarray(loss, dtype=np.float64)


# revision 18
# speedup vs baseline: 3.2395x; 1.1427x over previous
"""HEX loss kernel for Trainium2 (8 NeuronCores, batch-parallel, raw Bass).

Math: the chain junction-tree distribution factorizes into independent
Bernoullis with P(y_v=1) = sigmoid(fs[b,v]); hence
    loss = mean_b softplus(-fs[b, labels[b]])

Implementation: only fs[b, labels[b]] matters. Rows are assigned to
cores/slots on the host so that slot s = q*C + j holds a row whose
label lies in 32-column block q (8 blocks, C=640 slots each, padded
with duplicate rows). The device then needs just ONE diagonal strided
SWDGE cast-DMA that reads each slot's 32-float block (0.64 MB instead
of 4 MB per core), an additive host-built mask (+BIG on pad slots so
they contribute softplus=0, 0 at the label column, -BIG elsewhere) +
grouped reduce_max to pick the target, and softplus(-sel) =
Ln(1 + Exp(-sel)) on ACT with the free-dim sum from the activation
accumulator. The output store is not waited on (runtime quiesces DMA
queues at NEFF end; an explicit wait costs ~8 us of HBM-write receipt).
Host sums the 8x128 partials / B.
"""

import numpy as np

B = 32768
V = 256
N_CORES = 8
BL = B // N_CORES   # 4096 rows per core
P = 128
K = 32              # floats per block read per row
NQ = V // K         # 8 column blocks
C = 640             # padded slots per (core, block); 5*128
G = C // P          # 5
SLOTS = NQ * C      # 5120 slots per core
W = SLOTS * K // P  # 1280 free-dim elems per partition
SEL = SLOTS // P    # 40 selected values per partition
BIG = 1.0e30

_CACHE = {}


def _build():
    from contextlib import ExitStack

    import concourse.bass as bass
    import concourse.tile as tile  # noqa
    from concourse import bacc, mybir

    f32 = mybir.dt.float32
    bf16 = mybir.dt.bfloat16
    Act = mybir.ActivationFunctionType
    Alu = mybir.AluOpType

    nc = bacc.Bacc(
        "TRN2",
        target_bir_lowering=False,
        debug=False,
        enable_asserts=False,
        num_devices=N_CORES,
    )

    fsp_d = nc.dram_tensor("fsp", [SLOTS, V], f32, kind="ExternalInput").ap()
    msk_d = nc.dram_tensor("msk", [P, W], bf16, kind="ExternalInput").ap()
    out_d = nc.dram_tensor("out", [P, 1], f32, kind="ExternalOutput").ap()

    # diagonal views, one per g: element (p, q, k) = fsp[q*C + g*P + p, K*q + k]
    # (the DMA AP balancer allows at most 3 dims, so the g axis is unrolled)
    fs_diag = [
        bass.AP(
            fsp_d.tensor,
            g * P * V,
            [[V, P], [C * V + K, NQ], [1, K]],
        )
        for g in range(G)
    ]

    with ExitStack() as ctx:
        msk = ctx.enter_context(nc.sbuf_tensor([P, W], bf16))
        gath = ctx.enter_context(nc.sbuf_tensor([P, W], f32))
        t = ctx.enter_context(nc.sbuf_tensor([P, W], bf16))
        sel = ctx.enter_context(nc.sbuf_tensor([P, SEL], f32))
        u = ctx.enter_context(nc.sbuf_tensor([P, SEL], f32))
        y = ctx.enter_context(nc.sbuf_tensor([P, SEL], f32))
        acc = ctx.enter_context(nc.sbuf_tensor([P, 1], f32))

        sem_g = ctx.enter_context(nc.semaphore("s_g"))
        sem_m = ctx.enter_context(nc.semaphore("s_m"))
        sem_sel = ctx.enter_context(nc.semaphore("s_sel"))
        sem_out = ctx.enter_context(nc.semaphore("s_out"))

        blk = ctx.enter_context(nc.Block())

        gview = gath.ap().rearrange("p (q g k) -> p q g k", q=NQ, k=K)

        # bf16 view of the high u16 half of each f32 in gath (truncated
        # bf16) so both select operands run at bf16 DVE rates.
        gb = gath.ap().bitcast(bf16)
        gath_hi = bass.AP(gb.tensor, gb.offset + 1, [[2 * W, P], [2, W]])

        @blk.sync
        def _(s_eng):
            s_eng.dma_start(out=gview[:, :, 0, :], in_=fs_diag[0]).then_inc(sem_g, 16)
            s_eng.dma_start(out=gview[:, :, 1, :], in_=fs_diag[1]).then_inc(sem_g, 16)
            s_eng.dma_start(out=msk.ap(), in_=msk_d).then_inc(sem_m, 16)

        @blk.gpsimd
        def _(g_eng):
            g_eng.dma_start(out=gview[:, :, 4, :], in_=fs_diag[4]).then_inc(sem_g, 16)

        @blk.vector
        def _(v_eng):
            v_eng.wait_ge(sem_g, 16 * G)
            v_eng.wait_ge(sem_m, 16)
            v_eng.tensor_add(t.ap(), gath_hi, msk.ap())
            v_eng.drain()
            v_eng.tensor_reduce(
                sel.ap(),
                t.ap().rearrange("p (c k) -> p c k", k=K),
                axis=mybir.AxisListType.X,
                op=Alu.max,
            ).then_inc(sem_sel, 1)

        @blk.scalar
        def _(a_eng):
            from concourse.hw_specs import get_activation_tables

            a_eng.dma_start(out=gview[:, :, 2, :], in_=fs_diag[2]).then_inc(sem_g, 16)
            a_eng.dma_start(out=gview[:, :, 3, :], in_=fs_diag[3]).then_inc(sem_g, 16)
            tabs = list(get_activation_tables(nc.m.arch).items())
            tid = next(
                i for i, (n, s) in enumerate(tabs) if Act.Exp in s and Act.Ln in s
            )
            a_eng.add_instruction(
                mybir.InstLoadActFuncSet(
                    name=nc.get_next_instruction_name(),
                    ins=[],
                    outs=[],
                    act_func_set_id=tid,
                )
            )
            a_eng.wait_ge(sem_sel, 1)
            a_eng.activation(u.ap(), sel.ap(), Act.Exp, scale=-1.0)
            a_eng.drain()
            a_eng.activation(y.ap(), u.ap(), Act.Ln, bias=1.0, accum_out=acc.ap())
            a_eng.drain()
            # no wait on sem_out: the ~8us HBM-write receipt would sit on the
            # critical path; the runtime quiesces DMA queues at NEFF end.
            a_eng.dma_start(out=out_d, in_=acc.ap()).then_inc(sem_out, 16)

    nc.compile()
    return nc


def _get_nc():
    if "nc" not in _CACHE:
        _CACHE["nc"] = _build()
    return _CACHE["nc"]


def _shard_inputs(fs, labels):
    import ml_dtypes

    fs = np.ascontiguousarray(np.asarray(fs, dtype=np.float32))
    labels = np.asarray(labels).astype(np.int64)
    q_all = labels >> 5          # column block of each row
    kk_all = labels & (K - 1)    # position within the block

    # Assign rows to (core, block-bucket) with global balancing: rows of each
    # block q are dealt round-robin across cores, so every (core, q) bucket
    # holds <= ceil(count_q / 8) <= C rows.
    order = np.argsort(q_all, kind="stable")
    counts = np.bincount(q_all, minlength=NQ)
    assert counts.max() <= C * N_CORES, counts
    rows_by_cq = [[[] for _ in range(NQ)] for _ in range(N_CORES)]
    pos = 0
    for q in range(NQ):
        rows_q = order[pos : pos + counts[q]]
        pos += counts[q]
        for i, r in enumerate(rows_q):
            rows_by_cq[i % N_CORES][q].append(r)

    in_maps = []
    for c in range(N_CORES):
        slot_rows = np.zeros(SLOTS, dtype=np.int64)
        pad = np.ones(SLOTS, dtype=bool)
        for q in range(NQ):
            rows = rows_by_cq[c][q]
            n = len(rows)
            assert n <= C, (c, q, n)
            slot_rows[q * C : q * C + n] = rows
            pad[q * C : q * C + n] = False
        fsp = fs[slot_rows]  # [SLOTS, V]

        # mask in slot layout -> [P, W]: slot s = q*C + g*P + p maps to
        # partition p, free offset q*(G*K) + g*K
        kk = kk_all[slot_rows]
        m = np.full((SLOTS, K), -BIG, dtype=np.float32)
        m[np.arange(SLOTS), kk] = 0.0
        m[pad] = BIG
        msk = (
            m.reshape(NQ, G, P, K)
            .transpose(2, 0, 1, 3)
            .reshape(P, W)
            .astype(ml_dtypes.bfloat16)
        )
        in_maps.append(
            {"fsp": fsp, "msk": np.ascontiguousarray(msk)}
        )
    return in_maps


def kernel(fs, labels, _trace=False, _trace_kwargs=None):
    from concourse.bass_utils import run_bass_kernel_spmd

    nc = _get_nc()
    in_maps = _shard_inputs(fs, labels)
    res = run_bass_kernel_spmd(
        nc,
        in_maps,
        core_ids=list(range(N_CORES)),
        trace=_trace,
        **(_trace_kwargs or {}),
    )
    total = np.float64(0.0)
    for c in range(N_CORES):
        total += res.results[c]["out"].astype(np.float64).sum()
    loss = total / np.float64(B)
    if _trace:
        return np.float64(loss), res
    return np.asarray(loss, dtype=np.float64)


# revision 22
# speedup vs baseline: 3.5428x; 1.0936x over previous
"""HEX loss kernel for Trainium2 (8 NeuronCores, batch-parallel, raw Bass).

Math: the chain junction-tree distribution factorizes into independent
Bernoullis with P(y_v=1) = sigmoid(fs[b,v]); hence
    loss = mean_b softplus(-fs[b, labels[b]])

Implementation: only fs[b, labels[b]] matters. Rows are assigned to
cores/slots on the host so that slot s = q*C + j holds a row whose
label lies in 32-column block q (8 blocks, C=640 slots each, padded
with duplicate rows). The device then needs just ONE diagonal strided
SWDGE cast-DMA that reads each slot's 32-float block (0.64 MB instead
of 4 MB per core), an additive host-built mask (+BIG on pad slots so
they contribute softplus=0, 0 at the label column, -BIG elsewhere) +
grouped reduce_max to pick the target, and softplus(-sel) =
Ln(1 + Exp(-sel)) on ACT with the free-dim sum from the activation
accumulator. The output store is not waited on (runtime quiesces DMA
queues at NEFF end; an explicit wait costs ~8 us of HBM-write receipt).
Host sums the 8x128 partials / B.
"""

import numpy as np

B = 32768
V = 256
N_CORES = 8
BL = B // N_CORES   # 4096 rows per core
P = 128
K = 16              # floats per block read per row
NQ = V // K         # 16 column blocks
C = 384             # padded slots per (core, block); 3*128
G = C // P          # 3
SLOTS = NQ * C      # 6144 slots per core
W = SLOTS * K // P  # 768 free-dim elems per partition
SEL = SLOTS // P    # 48 selected values per partition
BIG = 1.0e30

_CACHE = {}


def _build():
    from contextlib import ExitStack

    import concourse.bass as bass
    import concourse.tile as tile  # noqa
    from concourse import bacc, mybir

    f32 = mybir.dt.float32
    bf16 = mybir.dt.bfloat16
    Act = mybir.ActivationFunctionType
    Alu = mybir.AluOpType

    nc = bacc.Bacc(
        "TRN2",
        target_bir_lowering=False,
        debug=False,
        enable_asserts=False,
        num_devices=N_CORES,
    )

    fsp_d = nc.dram_tensor("fsp", [SLOTS, V], f32, kind="ExternalInput").ap()
    msk_d = nc.dram_tensor("msk", [P, W], bf16, kind="ExternalInput").ap()
    out_d = nc.dram_tensor("out", [P, 1], f32, kind="ExternalOutput").ap()

    # diagonal views, one per g: element (p, q, k) = fsp[q*C + g*P + p, K*q + k]
    # (the DMA AP balancer allows at most 3 dims, so the g axis is unrolled)
    fs_diag = [
        bass.AP(
            fsp_d.tensor,
            g * P * V,
            [[V, P], [C * V + K, NQ], [1, K]],
        )
        for g in range(G)
    ]

    with ExitStack() as ctx:
        msk = ctx.enter_context(nc.sbuf_tensor([P, W], bf16))
        gath = ctx.enter_context(nc.sbuf_tensor([P, W], f32))
        t = ctx.enter_context(nc.sbuf_tensor([P, W], bf16))
        sel = ctx.enter_context(nc.sbuf_tensor([P, SEL], f32))
        u = ctx.enter_context(nc.sbuf_tensor([P, SEL], f32))
        y = ctx.enter_context(nc.sbuf_tensor([P, SEL], f32))
        acc = ctx.enter_context(nc.sbuf_tensor([P, 1], f32))

        sem_g = ctx.enter_context(nc.semaphore("s_g"))
        sem_m = ctx.enter_context(nc.semaphore("s_m"))
        sem_sel = ctx.enter_context(nc.semaphore("s_sel"))
        sem_out = ctx.enter_context(nc.semaphore("s_out"))

        blk = ctx.enter_context(nc.Block())

        gview = gath.ap().rearrange("p (q g k) -> p q g k", q=NQ, k=K)

        # bf16 view of the high u16 half of each f32 in gath (truncated
        # bf16) so both select operands run at bf16 DVE rates.
        gb = gath.ap().bitcast(bf16)
        gath_hi = bass.AP(gb.tensor, gb.offset + 1, [[2 * W, P], [2, W]])

        @blk.sync
        def _(s_eng):
            s_eng.dma_start(out=gview[:, :, 0, :], in_=fs_diag[0]).then_inc(sem_g, 16)
            s_eng.dma_start(out=msk.ap(), in_=msk_d).then_inc(sem_m, 16)

        @blk.vector
        def _(v_eng):
            v_eng.wait_ge(sem_g, 16 * G)
            v_eng.wait_ge(sem_m, 16)
            v_eng.tensor_add(t.ap(), gath_hi, msk.ap())
            v_eng.drain()
            v_eng.tensor_reduce(
                sel.ap(),
                t.ap().rearrange("p (c k) -> p c k", k=K),
                axis=mybir.AxisListType.X,
                op=Alu.max,
            ).then_inc(sem_sel, 1)

        @blk.scalar
        def _(a_eng):
            from concourse.hw_specs import get_activation_tables

            tabs = list(get_activation_tables(nc.m.arch).items())
            tid = next(
                i for i, (n, s) in enumerate(tabs) if Act.Exp in s and Act.Ln in s
            )
            a_eng.add_instruction(
                mybir.InstLoadActFuncSet(
                    name=nc.get_next_instruction_name(),
                    ins=[],
                    outs=[],
                    act_func_set_id=tid,
                )
            )
            a_eng.dma_start(out=gview[:, :, 1, :], in_=fs_diag[1]).then_inc(sem_g, 16)
            a_eng.dma_start(out=gview[:, :, 2, :], in_=fs_diag[2]).then_inc(sem_g, 16)
            a_eng.wait_ge(sem_sel, 1)
            a_eng.activation(u.ap(), sel.ap(), Act.Exp, scale=-1.0)
            a_eng.drain()
            a_eng.activation(y.ap(), u.ap(), Act.Ln, bias=1.0, accum_out=acc.ap())
            a_eng.drain()
            # no wait on sem_out: the ~8us HBM-write receipt would sit on the
            # critical path; the runtime quiesces DMA queues at NEFF end.
            a_eng.dma_start(out=out_d, in_=acc.ap()).then_inc(sem_out, 16)

    nc.compile()
    return nc


def _get_nc():
    if "nc" not in _CACHE:
        _CACHE["nc"] = _build()
    return _CACHE["nc"]


def _shard_inputs(fs, labels):
    import ml_dtypes

    fs = np.ascontiguousarray(np.asarray(fs, dtype=np.float32))
    labels = np.asarray(labels).astype(np.int64)
    q_all = labels // K          # column block of each row
    kk_all = labels % K          # position within the block

    # Assign rows to (core, block-bucket) with global balancing: rows of each
    # block q are dealt round-robin across cores, so every (core, q) bucket
    # holds <= ceil(count_q / 8) <= C rows.
    order = np.argsort(q_all, kind="stable")
    counts = np.bincount(q_all, minlength=NQ)
    assert counts.max() <= C * N_CORES, counts
    rows_by_cq = [[[] for _ in range(NQ)] for _ in range(N_CORES)]
    pos = 0
    for q in range(NQ):
        rows_q = order[pos : pos + counts[q]]
        pos += counts[q]
        for i, r in enumerate(rows_q):
            rows_by_cq[i % N_CORES][q].append(r)

    in_maps = []
    for c in range(N_CORES):
        slot_rows = np.zeros(SLOTS, dtype=np.int64)
        pad = np.ones(SLOTS, dtype=bool)
        for q in range(NQ):
            rows = rows_by_cq[c][q]
            n = len(rows)
            assert n <= C, (c, q, n)
            slot_rows[q * C : q * C + n] = rows
            pad[q * C : q * C + n] = False
        fsp = fs[slot_rows]  # [SLOTS, V]

        # mask in slot layout -> [P, W]: slot s = q*C + g*P + p maps to
        # partition p, free offset q*(G*K) + g*K
        kk = kk_all[slot_rows]
        m = np.full((SLOTS, K), -BIG, dtype=np.float32)
        m[np.arange(SLOTS), kk] = 0.0
        m[pad] = BIG
        msk = (
            m.reshape(NQ, G, P, K)
            .transpose(2, 0, 1, 3)
            .reshape(P, W)
            .astype(ml_dtypes.bfloat16)
        )
        in_maps.append(
            {"fsp": fsp, "msk": np.ascontiguousarray(msk)}
        )
    return in_maps


def kernel(fs, labels, _trace=False, _trace_kwargs=None):
    from concourse.bass_utils import run_bass_kernel_spmd

    nc = _get_nc()
    in_maps = _shard_inputs(fs, labels)
    res = run_bass_kernel_spmd(
        nc,
        in_maps,
        core_ids=list(range(N_CORES)),
        trace=_trace,
        **(_trace_kwargs or {}),
    )
    total = np.float64(0.0)
    for c in range(N_CORES):
        total += res.results[c]["out"].astype(np.float64).sum()
    loss = total / np.float64(B)
    if _trace:
        return np.float64(loss), res
    return np.asarray(loss, dtype=np.float64)
